# revision 42
# baseline (speedup 1.0000x reference)
"""Causal multi-head attention (B=2, T=2048, D=1024, H=16) on 8 TRN2 NeuronCores.

Sharding: core c owns heads {2c, 2c+1} (= 128 contiguous dims of D) of BOTH
batches — head-parallel over all 8 cores, batch handled inside each core.
This makes the output-projection exchange a single 8-core AllToAll per q-span
of the (normalized, bf16) attention outputs: shard j of core c's send buffer
is its yT slice for (batch j//4, q-tile j%4), and received slot i is D-chunk
i for the core's own (batch, q-tile) = (c//4, c%4). Every access pattern in
that exchange is core-independent, so one SPMD program serves all 8 cores,
and the wire traffic is ~1MB bf16 total (vs ReduceScattering 8MB of fp32
partial outputs per core). Each core then computes the full-D out-projection
for its q-tile; the output bias is added on the host.

Device-side layout (host pre-transposes, pure data movement):
  - xT  [2, D, T]     = x[b].T so projections contract D on the partition
                        dim; span-0/1 slices are DMA'd first so the first
                        projection never waits for the full 8MB.
  - qT/kT [b][128, T] computed directly transposed (dims on partitions);
                        the core's 2 heads at partitions 0-63 / 64-127.
  - scoresT[k, q]     = k @ qT; the two heads are computed by two row-tiled
                        matmuls (tile_position (0,0)/(64,0), K=64 each) that
                        run concurrently in the PE array, writing two
                        adjacent PSUM banks.
  - exp               one ScalarE activation per k-tile covers both heads'
                        scores ([128, 1024] across the 2 banks); the scores
                        of k-tile t+1 are issued before the AV of k-tile t so
                        the PE queue never parks on an exp. Diagonal tiles
                        trim the leading fully-masked columns from the scores
                        matmul, the exp, and the AV matmul; mask values are
                        applied only on the [128, 128] triangle blocks.
  - AV                col-tiled pack: both heads' [128 k, 64] v blocks run
                        concurrently (out partitions 0-63 / 64-127 of one
                        PSUM bank). The softmax denominators accumulate in a
                        second bank via rank-1 col-tiled matmuls, 2 k-tiles
                        per 4-matmul pack (rows {0,64} even / {32,96} odd).
  - normalization     one rank-97 selector matmul sums each head's even/odd
                        denominator rows and broadcasts the total across the
                        head's partitions; a fast-approx reciprocal (DVE
                        custom op, ~18 bits) and one multiply normalize yT.
  - out-projection    all issued after the attention stream (the PE FIFO
                        must never park behind an in-flight collective):
                        8 accumulating matmuls per [128 q, 512] tile over the
                        AllToAll-gathered full-D yT, ScalarE PSUM evacuation,
                        DMA straight to the output.

Dtypes: all matmul operands bf16 with fp32 PSUM accumulation; exp and the
normalization run in fp32. ScalarE does (almost) nothing but exp; the PE is
kept warm with a short warm-up matmul burst and projection blocks interleaved
between attention blocks. Measured end-to-end relative error ~5.9e-3 vs the
fp32 reference; HW exec time ~230-265us (run-to-run variance is dominated by
cross-core execution-start skew absorbed by the first collective barrier).
"""

import os
import numpy as np
import ml_dtypes

BF16 = ml_dtypes.bfloat16

B, T, D, H = 2, 2048, 1024, 16
HD = D // H                     # 64
NCORES = 8
DL = D // NCORES                # dims per core = 128 (2 heads)
SP = 512                        # free-dim span per matmul (one PSUM bank, fp32)
QS = T // SP                    # 4 q spans
KT = T // 128                   # 16 k tiles
SCALE = HD ** -0.5

_CACHE = {}


def _build_program():
    import concourse.bass as bass  # noqa: F401  (registers bass machinery)
    import concourse.tile as tile
    from concourse import bacc, mybir

    f32 = mybir.dt.float32
    f32r = mybir.dt.float32r
    bf16 = mybir.dt.bfloat16
    Exp = mybir.ActivationFunctionType.Exp

    nc = bacc.Bacc("TRN2", target_bir_lowering=False, debug=False,
                   num_devices=NCORES)

    xT = nc.dram_tensor("xT", [B, D, T], bf16, kind="ExternalInput")
    wqT = nc.dram_tensor("wqT", [D, DL], bf16, kind="ExternalInput")
    wkT = nc.dram_tensor("wkT", [D, DL], bf16, kind="ExternalInput")
    wvT = nc.dram_tensor("wvT", [D, DL], bf16, kind="ExternalInput")
    woT = nc.dram_tensor("woT", [D, D], bf16, kind="ExternalInput")
    bqP = nc.dram_tensor("bqP", [128, 1], f32, kind="ExternalInput")
    bkP = nc.dram_tensor("bkP", [128, 1], f32, kind="ExternalInput")
    bv = nc.dram_tensor("bv", [1, DL], bf16, kind="ExternalInput")
    mtriD = nc.dram_tensor("mtriD", [128, B * 128], bf16,
                           kind="ExternalInput")
    out_ext = nc.dram_tensor("out", [QS, 128, D], bf16,
                             kind="ExternalOutput")

    RG = [[0, 1, 2, 3, 4, 5, 6, 7]]

    with tile.TileContext(nc) as tc:
        with tc.tile_pool(name="main", bufs=1) as main, \
             tc.tile_pool(name="dram", bufs=1, space="DRAM") as dram:
            xt_s = main.tile([128, B, 8, T], bf16)
            wq_s = main.tile([128, 8, DL], bf16)
            wk_s = main.tile([128, 8, DL], bf16)
            wv_s = main.tile([128, 8, DL], bf16)
            woT_s = main.tile([128, 8, D], bf16)
            qT_s = main.tile([128, B, T], bf16)
            kT_s = main.tile([128, B, T], bf16)
            yT_s = main.tile([128, B, T], bf16)
            v_s = main.tile([128, B, KT, 128], bf16)
            onesP = main.tile([128, 1], bf16)
            bq_s = main.tile([128, 1], f32)
            bk_s = main.tile([128, 1], f32)
            bv_bc = main.tile([128, DL], bf16)
            mtri_s = main.tile([128, B, 128], bf16)
            # selector for the denominator sum+broadcast: one matmul
            # rb_den = sel4.T @ den_stack adds each head's even/odd k-tile
            # partial denominators (rows 0/32 for head A, 64/96 for head B)
            # and broadcasts the total across that head's 64 partitions
            sel4_s = main.tile([97, 128], bf16)
            den_sb_all = main.tile([97, B * QS, SP], bf16)
            warm_s = main.tile([128, SP], bf16)
            dum_o = main.tile([1, 2], bf16)

            a2a_in = [dram.tile([NCORES * 128, 128], bf16, name=f"a2ai{s}")
                      for s in range(QS)]
            a2a_out = [dram.tile([NCORES * 128, 128], bf16, name=f"a2ao{s}")
                       for s in range(QS)]
            # constants (DVE) + ACT table warm-up before any real dependency
            nc.vector.memset(warm_s, 0.25)
            nc.vector.memset(onesP, 1.0)
            nc.vector.memset(den_sb_all, 0.0)
            nc.vector.memset(sel4_s, 0.0)
            nc.vector.memset(sel4_s[0:1, 0:64], 1.0)
            nc.vector.memset(sel4_s[32:33, 0:64], 1.0)
            nc.vector.memset(sel4_s[64:65, 64:128], 1.0)
            nc.vector.memset(sel4_s[96:97, 64:128], 1.0)
            nc.scalar.activation(dum_o, warm_s[0:1, 0:2], Exp)

            # loads spread over four engine DMA queues; batch-0 x + the
            # qkv weights land first, batch-1 x next, bulk (woT) last
            engs = [nc.sync, nc.gpsimd, nc.scalar]
            nc.sync.dma_start(out=bq_s, in_=bqP[:])
            nc.sync.dma_start(out=bk_s, in_=bkP[:])
            wq_r = wqT[:].rearrange("(c p) n -> c p n", p=128)
            for c in range(8):
                nc.sync.dma_start(out=wq_s[:, c, :], in_=wq_r[c])
            xT_r = xT[:].rearrange("b (c p) t -> b c p t", p=128)

            def x_span(b, s):
                for c in range(8):
                    engs[c % 3].dma_start(
                        out=xt_s[:, b, c, s * SP:(s + 1) * SP],
                        in_=xT_r[b, c][:, s * SP:(s + 1) * SP])

            # span-0 slices of batch 0 land in ~3us so the first projection
            # block never waits for the full 8MB of x
            x_span(0, 0)
            for w_s, w_d in ((wk_s, wkT), (wv_s, wvT)):
                w_r = w_d[:].rearrange("(c p) n -> c p n", p=128)
                for c in range(8):
                    nc.gpsimd.dma_start(out=w_s[:, c, :], in_=w_r[c])
            nc.scalar.dma_start(
                out=mtri_s[:].rearrange("p b q -> p (b q)"), in_=mtriD[:])
            x_span(1, 0)
            x_span(0, 1)
            x_span(1, 1)
            for b in range(B):
                for c in range(8):
                    engs[c % 3].dma_start(
                        out=xt_s[:, b, c, 2 * SP:],
                        in_=xT_r[b, c][:, 2 * SP:])
            nc.scalar.dma_start(out=bv_bc, in_=bv[:].to_broadcast([128, DL]))
            woT_r = woT[:].rearrange("(c p) n -> c p n", p=128)
            for c in range(8):
                engs[c % 3].dma_start(out=woT_s[:, c, :], in_=woT_r[c])

            with tc.tile_pool(name="sc_psum", bufs=2, space="PSUM") as sc_psum, \
                 tc.tile_pool(name="av_psum", bufs=1, space="PSUM") as av_psum, \
                 tc.tile_pool(name="mm_psum", bufs=2, space="PSUM") as mm_psum, \
                 tc.tile_pool(name="at_sb", bufs=6) as at_sb, \
                 tc.tile_pool(name="ytf_sb", bufs=4) as ytf_sb, \
                 tc.tile_pool(name="ob_sb", bufs=3) as ob_sb:

                # PE warm-up during the initial DMA wait: gets the HAM clock
                # gate to 8/8 before the first projection matmul
                def warm(n):
                    for i in range(n):
                        wm = mm_psum.tile([128, SP], f32, tag="mm")
                        nc.tensor.matmul(wm, lhsT=warm_s[:, 0:128],
                                         rhs=warm_s, start=True, stop=True)
                warm(8)

                def proj_block(sp, b):
                    # q/k for span sp and v for k-tiles 4sp..4sp+3 of batch b
                    for w_s, b_s, dst in ((wq_s, bq_s, qT_s),
                                          (wk_s, bk_s, kT_s)):
                        ps = mm_psum.tile([128, SP], f32, tag="mm")
                        for kc in range(8):
                            nc.tensor.matmul(
                                ps,
                                lhsT=w_s[:, kc, :],
                                rhs=xt_s[:, b, kc, sp * SP:(sp + 1) * SP],
                                start=(kc == 0), stop=(kc == 7))
                        nc.vector.tensor_scalar_add(
                            dst[:, b, sp * SP:(sp + 1) * SP], ps, b_s)
                    for mt in range(4 * sp, 4 * sp + 4):
                        ps = mm_psum.tile([128, SP], f32, tag="mm")
                        for kc in range(8):
                            nc.tensor.matmul(
                                ps[:, 0:DL],
                                lhsT=xt_s[:, b, kc,
                                          mt * 128:(mt + 1) * 128],
                                rhs=wv_s[:, kc, :],
                                start=(kc == 0), stop=(kc == 7))
                        nc.vector.tensor_add(v_s[:, b, mt, :],
                                             ps[:, 0:DL], bv_bc)

                def proj_work(sp, b):
                    # the projection block as ~4-matmul chunks that the
                    # attention k-tile loop interleaves into the PE stream
                    work = []
                    state = {}

                    def qk_chunk(w_s, b_s, dst, half):
                        def run():
                            key = (id(w_s), sp, b)
                            if half == 0:
                                state[key] = mm_psum.tile(
                                    [128, SP], f32, tag="mm", name="psf")
                            ps = state[key]
                            for kc in range(4 * half, 4 * half + 4):
                                nc.tensor.matmul(
                                    ps, lhsT=w_s[:, kc, :],
                                    rhs=xt_s[:, b, kc,
                                             sp * SP:(sp + 1) * SP],
                                    start=(kc == 0), stop=(kc == 7))
                            if half == 1:
                                nc.vector.tensor_scalar_add(
                                    dst[:, b, sp * SP:(sp + 1) * SP],
                                    state.pop(key), b_s)
                        return run

                    def v_chunk(mt, half):
                        def run():
                            key = ("v", mt, b)
                            if half == 0:
                                state[key] = mm_psum.tile(
                                    [128, SP], f32, tag="mm", name="psf")
                            ps = state[key]
                            for kc in range(4 * half, 4 * half + 4):
                                nc.tensor.matmul(
                                    ps[:, 0:DL],
                                    lhsT=xt_s[:, b, kc,
                                              mt * 128:(mt + 1) * 128],
                                    rhs=wv_s[:, kc, :],
                                    start=(kc == 0), stop=(kc == 7))
                            if half == 1:
                                nc.vector.tensor_add(
                                    v_s[:, b, mt, :],
                                    state.pop(key)[:, 0:DL], bv_bc)
                        return run

                    for w_s, b_s, dst in ((wq_s, bq_s, qT_s),
                                          (wk_s, bk_s, kT_s)):
                        work.append(qk_chunk(w_s, b_s, dst, 0))
                        work.append(qk_chunk(w_s, b_s, dst, 1))
                    for mt in range(4 * sp, 4 * sp + 4):
                        work.append(v_chunk(mt, 0))
                        work.append(v_chunk(mt, 1))
                    return work

                def attn(sp, b, fillers=None):
                    # both heads for batch b; returns the rec slot
                    nkt = 4 * sp + 4
                    av = av_psum.tile([128, SP], f32, tag="av")
                    den = av_psum.tile([97, SP], f32, tag="den")

                    def sc_exp(kt):
                        # scores (row-tiled pair) + exp + triangle mask
                        c0 = max(0, 128 * (kt - 4 * sp))
                        sc = sc_psum.tile([128, 2 * SP], f32, tag="sc")
                        for hh in range(2):
                            r0 = 64 * hh
                            nc.tensor.matmul(
                                sc[:, hh * SP + c0:(hh + 1) * SP],
                                lhsT=kT_s[r0:r0 + 64, b,
                                          kt * 128:(kt + 1) * 128],
                                rhs=qT_s[r0:r0 + 64, b,
                                         sp * SP + c0:(sp + 1) * SP],
                                start=True, stop=True)
                        at = at_sb.tile([128, 2 * SP], bf16, tag="at")
                        if c0:
                            nc.scalar.activation(
                                at.rearrange("p (g q) -> p g q",
                                             g=2)[:, :, c0:],
                                sc.rearrange("p (g q) -> p g q",
                                             g=2)[:, :, c0:],
                                Exp)
                        else:
                            nc.scalar.activation(at, sc, Exp)
                        if kt >= 4 * sp:  # diagonal tile: mask the triangle
                            for hh in range(2):
                                blk = at[:, hh * SP + c0:hh * SP + c0 + 128]
                                nc.vector.tensor_mul(blk, blk,
                                                     mtri_s[:, b, :])
                        return at, c0

                    # software-pipelined: the k-tile after next's scores are
                    # already in the PE queue when an AV waits on its exp.
                    # AV runs both heads as one col-tiled pack (out partitions
                    # 0-63 / 64-127 of one bank); the denominator rows are
                    # accumulated by rank-1 col-tiled matmuls, 2 k-tiles per
                    # pack at partitions {0,64} (kt even) / {32,96} (kt odd)
                    pend = {0: sc_exp(0)}
                    hold = {}
                    fillers = list(fillers or [])
                    npop = -(-len(fillers) // nkt) if fillers else 0
                    for kt in range(nkt):
                        if kt + 1 < nkt:
                            pend[kt + 1] = sc_exp(kt + 1)
                        hold[kt] = pend.pop(kt)
                        at, c0 = hold[kt]
                        for hh in range(2):
                            nc.tensor.matmul(
                                av[64 * hh:64 * (hh + 1), c0:SP],
                                lhsT=v_s[:, b, kt, 64 * hh:64 * (hh + 1)],
                                rhs=at[:, hh * SP + c0:(hh + 1) * SP],
                                start=(kt == 0), stop=(kt == nkt - 1))
                        if kt % 2 == 1:
                            # 4 adjacent rank-1 matmuls -> one col-tiled pack
                            for kk in (kt - 1, kt):
                                att, cc0 = hold.pop(kk)
                                r0 = 32 * (kk % 2)
                                for hh in range(2):
                                    nc.tensor.matmul(
                                        den[r0 + 64 * hh:
                                            r0 + 64 * hh + 1, cc0:SP],
                                        lhsT=onesP,
                                        rhs=att[:, hh * SP + cc0:
                                                (hh + 1) * SP],
                                        start=(kt == 1), stop=(kt == nkt - 1),
                                        tile_position=(0, r0 + 64 * hh))
                        for _ in range(min(npop, len(fillers))):
                            fillers.pop(0)()
                    for f in fillers:
                        f()
                    den_sb = den_sb_all[:, B * sp + b, :]
                    for r in (0, 32, 64, 96):
                        # odd-parity rows of span 0 are only written from
                        # col 128 (diagonal trim); the rest stays zero
                        lo = 128 if (sp == 0 and r in (32, 96)) else 0
                        nc.vector.tensor_copy(den_sb[r:r + 1, lo:],
                                              den[r:r + 1, lo:])
                    nc.vector.tensor_copy(yT_s[:, b, sp * SP:(sp + 1) * SP],
                                          av)
                    return den_sb

                def post(sp, b, den_sb):
                    # sum + broadcast the denominators across partitions via
                    # one rank-97 selector matmul, reciprocal on DVE, then
                    # normalize yT in place
                    rb = mm_psum.tile([128, SP], f32, tag="mm")
                    nc.tensor.matmul(rb, lhsT=sel4_s, rhs=den_sb,
                                     start=True, stop=True)
                    rbf = ob_sb.tile([128, SP], f32, tag="ob")
                    nc.vector.tensor_copy(rbf, rb)
                    rbr = ob_sb.tile([128, SP], f32, tag="ob")
                    nc.vector.reciprocal_approx_fast(out=rbr, in_=rbf)
                    yv = yT_s[:, b, sp * SP:(sp + 1) * SP]
                    nc.vector.tensor_mul(yv, yv, rbr)

                def exchange(sp):
                    # shard j = my yT slice for (batch j//4, q-tile j%4);
                    # slot i of the output = D-chunk i of my own q-tile
                    for b in range(B):
                        for t in range(QS):
                            j = QS * b + t
                            nc.sync.dma_start(
                                out=a2a_in[sp][j * 128:(j + 1) * 128, :],
                                in_=yT_s[:, b, sp * SP + t * 128:
                                         sp * SP + (t + 1) * 128])
                    nc.gpsimd.collective_compute(
                        "AllToAll", mybir.AluOpType.bypass,
                        replica_groups=RG,
                        ins=[a2a_in[sp][:].opt()],
                        outs=[a2a_out[sp][:].opt()])
                    ytf = ytf_sb.tile([128, 8, 128], bf16, tag="ytf")
                    nc.sync.dma_start(
                        out=ytf,
                        in_=a2a_out[sp][:].rearrange("(i p) q -> p i q",
                                                     p=128))
                    return ytf

                def outproj(sp, ytf):
                    # full-D out-projection for this core's q-tile of span
                    # sp; output bias is added on the host
                    for ns in range(2):
                        po = mm_psum.tile([128, SP], f32, tag="mm")
                        for i in range(8):
                            nc.tensor.matmul(
                                po,
                                lhsT=ytf[:, i, :],
                                rhs=woT_s[:, i, ns * SP:(ns + 1) * SP],
                                start=(i == 0), stop=(i == 7))
                        ob = ob_sb.tile([128, SP], bf16, tag="ob")
                        nc.scalar.copy(ob, po)
                        eng = nc.sync if ns == 0 else nc.gpsimd
                        eng.dma_start(
                            out=out_ext[sp, :, ns * SP:(ns + 1) * SP], in_=ob)

                # software pipeline: post()/exchange()/outproj() are issued
                # behind later attention blocks so their PE work (which waits
                # on DVE/collective results) never stalls the PE queue
                recs = {}
                ytfs = {}
                proj_block(0, 0)
                recs[(0, 0)] = attn(0, 0, proj_work(0, 1))
                recs[(0, 1)] = attn(0, 1, proj_work(1, 0))
                post(0, 0, recs[(0, 0)])
                recs[(1, 0)] = attn(1, 0, proj_work(1, 1))
                post(0, 1, recs[(0, 1)])
                ytfs[0] = exchange(0)
                recs[(1, 1)] = attn(1, 1, proj_work(2, 0))
                post(1, 0, recs[(1, 0)])
                recs[(2, 0)] = attn(2, 0, proj_work(2, 1))
                post(1, 1, recs[(1, 1)])
                ytfs[1] = exchange(1)
                recs[(2, 1)] = attn(2, 1, proj_work(3, 0))
                post(2, 0, recs[(2, 0)])
                recs[(3, 0)] = attn(3, 0, proj_work(3, 1))
                post(2, 1, recs[(2, 1)])
                ytfs[2] = exchange(2)
                recs[(3, 1)] = attn(3, 1)
                post(3, 0, recs[(3, 0)])
                warm(4)
                post(3, 1, recs[(3, 1)])
                ytfs[3] = exchange(3)
                # all out-projections strictly last: the PE FIFO must never
                # park behind an in-flight collective. A few warm matmuls
                # during the final exchange wait keep the HAM clock gate
                # open for the last out-projection
                for sp in range(QS - 1):
                    outproj(sp, ytfs[sp])
                warm(6)
                outproj(QS - 1, ytfs[QS - 1])

    nc.compile()
    return nc


def _get_program():
    if "nc" not in _CACHE:
        _CACHE["nc"] = _build_program()
    return _CACHE["nc"]


def _make_in_maps(x, mask, Wq, bq, Wk, bk, Wv, bv, Wo, bo):
    x = np.asarray(x, np.float32)
    mask = np.asarray(mask, bool)
    Wq = np.asarray(Wq, np.float32)
    Wk = np.asarray(Wk, np.float32)
    Wv = np.asarray(Wv, np.float32)
    Wo = np.asarray(Wo, np.float32)
    bq = np.asarray(bq, np.float32)
    bk = np.asarray(bk, np.float32)
    bv = np.asarray(bv, np.float32)
    bo = np.asarray(bo, np.float32)

    xTd = np.ascontiguousarray(x.transpose(0, 2, 1)).astype(BF16)  # [B, D, T]
    woT = np.ascontiguousarray(Wo.T).astype(BF16)
    # the diagonal [128,128] triangle block of mask[b,0].T (k on rows;
    # identical on every diagonal tile of a causal mask), partition-major
    md = np.empty((B, 128, 128), np.float32)
    for b in range(B):
        md[b] = mask[b, 0].T[0:128, 0:128]
    md = np.ascontiguousarray(
        md.transpose(1, 0, 2)).reshape(128, B * 128).astype(BF16)

    in_maps = []
    for c in range(NCORES):
        sl = slice(c * DL, (c + 1) * DL)  # dims of heads {2c, 2c+1}
        in_maps.append({
            "xT": xTd,
            "wqT": np.ascontiguousarray((Wq[sl] * SCALE).T).astype(BF16),
            "wkT": np.ascontiguousarray(Wk[sl].T).astype(BF16),
            "wvT": np.ascontiguousarray(Wv[sl].T).astype(BF16),
            "woT": woT,
            "bqP": np.ascontiguousarray((bq[sl] * SCALE).reshape(DL, 1)),
            "bkP": np.ascontiguousarray(bk[sl].reshape(DL, 1)),
            "bv": bv[sl].reshape(1, DL).astype(BF16),
            "mtriD": md,
        })
    return in_maps


def _capture_profile(nc, in_maps, tmpdir):
    """Run with NTFF capture and process the profile ourselves (the stock
    trace path can't handle the duplicate-executable NTFFs the axon relay
    produces). Returns (results, exec_time_ns|None)."""
    import glob
    import json
    import re
    import subprocess
    from trn_agent_boot.trn_boot import _ntff_profile_via_ctypes
    from concourse import bass2jax

    hook = _ntff_profile_via_ctypes("/opt/axon/libaxon_pjrt.so")
    if hook is None:
        raise RuntimeError("libaxon_pjrt.so lacks NTFF profile symbols")
    os.makedirs(tmpdir, exist_ok=True)
    with hook(tmpdir, [0]):
        results = bass2jax.run_bass_via_pjrt(nc, in_maps, n_cores=NCORES)

    # group NTFF/NEFF pairs by executable id; use the newest executable
    ntffs = glob.glob(os.path.join(tmpdir, "*_body*-device*.ntff"))
    best, best_id = None, -1
    for f in ntffs:
        m = re.search(r"executable(\d+)-device000000", f)
        if m and int(m.group(1)) > best_id:
            best_id, best = int(m.group(1)), f
    if best is None:
        raise RuntimeError(f"no NTFF produced in {tmpdir}")
    neff = re.sub(r"-device\d+-execution-\d+\.ntff$", ".neff", best)
    out_json = os.path.join(tmpdir, "prof.json")
    subprocess.check_call(
        ["neuron-profile", "view", "--ignore-nc-buf-usage", "-s", best,
         "-n", neff, "--output-format=json", f"--output-file={out_json}"],
        cwd=tmpdir)
    summary = json.load(open(out_json))["summary"][0]
    return results, int(summary["total_time"] * 1e9)


def kernel(x, mask, Wq, bq, Wk, bk, Wv, bv, Wo, bo):
    from concourse import bass_utils

    in_maps = _make_in_maps(x, mask, Wq, bq, Wk, bk, Wv, bv, Wo, bo)
    nc = _get_program()

    trace = bool(int(os.environ.get("MHA_TRACE", "0")))
    tmpdir = os.environ.get("MHA_TRACE_DIR") or None
    results = None
    if trace and tmpdir:
        try:
            results, exec_ns = _capture_profile(nc, in_maps, tmpdir)
            _CACHE["last_exec_time_ns"] = exec_ns
        except Exception as e:  # profiling is best-effort
            print(f"profiling unavailable: {type(e).__name__}: {e}")
            results = None
    if results is None:
        results = bass_utils.run_bass_kernel_spmd(
            nc, in_maps, core_ids=list(range(NCORES))).results
        _CACHE.setdefault("last_exec_time_ns", None)

    out = np.empty((B, T, D), np.float32)
    for c in range(NCORES):
        b, t = divmod(c, QS)  # core c owns (batch b, q-tile t) of every span
        o = np.asarray(results[c]["out"], np.float32)
        for sp in range(QS):
            lo = sp * SP + t * 128
            out[b, lo:lo + 128] = o[sp]
    out += np.asarray(bo, np.float32).reshape(1, 1, D)
    return out
